# revision 42
# baseline (speedup 1.0000x reference)
"""Trainium2 Bass kernel for a 2-layer GCN (PyG GCNConv semantics) on 8 NeuronCores.

Design. The dominant cost is the per-edge dma_gather: Q7 SWDGE descriptor
generation runs at ~7.9ns/index per queue, but 4 SWDGE queues (queue_num
round-robin + num_swdge_queues=4) overlap generation to ~2.5ns/index,
leaving the 16 SDMA engines' 256B-descriptor drain as the floor:

  - nodes sharded 12500/core (12544 padded = 98 tiles of 128 rows); edges
    partitioned by dst core, grouped by (dst-block of 8 tiles, src-group),
    sorted by dst tile within each (block, group) span
  - spans are chunked into 128-slot chunks with NO per-tile alignment: a
    chunk at a tile boundary is consumed by both tiles (each with its own
    host-precomputed one-hot P slice; other tiles' slots are zero rows).
    Pads are gathered idx-0 rows with zero P rows (HW-measured: trailing -1
    pads desc-gen SLOWER, and runtime num_idxs_reg wedges the device)
  - gathers split at 4096 idx: larger instructions stall the SWDGE ring
  - self-loops never occupy gather slots (their src group depends on the
    core, which inflates the max-over-cores span pad AND they need no
    gather): applied per tile as one PE matmul of the bulk-loaded own-shard
    tile against an identity rhs, accumulated into the same PSUM chain
  - per chunk: PE matmul accumulates agg^T[feat, dstrow] into PSUM
  - per dst-tile: agg^T @ W^T; dinv[dst] applied post-matmul on ACT
  - norm separability: dinv[src] pre-scaled into gather rows, dinv[dst]
    post-applied (layer 1 uses dinv^2: the extra dinv is layer 2's source
    factor)
  - AllGather of y1 split in two halves: the first is issued mid-layer-1
    and re-assembled into y1_full by contiguous DRAM copies, so only half
    the collective sits on the layer boundary

Engine discipline: no DVE tensor_scalar/copy (2-port perf mode blocks the
SWDGE descriptor generation that dominates runtime). PSUM evacuation +
scaling on ACT; adds on DVE tensor_tensor (single-port)."""
import sys

sys.path.insert(0, "/opt/trn_rl_repo")

import numpy as np

N = 100000
E = 1600000
D = 128
CORES = 8
S = 12500          # real nodes per core
TPC = 98           # dst tiles per core
SP = TPC * 128     # padded nodes per core (12544)
NP = CORES * SP    # padded global rows (100352)
GRP = 32768        # src-group width (int16 gather-index limit)
NGRP = 4
BLK = 8            # dst tiles per block
NB = (TPC + BLK - 1) // BLK
PAD_NEG = False    # HW-measured: trailing -1 pads cost MORE gen time than
                   # gathered idx-0 pads, and value_load num_idxs_reg wedges
                   # the device -- so pads are idx 0, gathered, zero P rows
SPLIT = 16         # chunks per gather instruction (2048 idx): finer
                   # round-robin over the 4 SWDGE queues


def _build_schedule(src, dst):
    core = dst // S
    dl = dst % S
    t = dl >> 7
    r = dl & 127
    b = t // BLK
    sp = (src // S) * SP + (src % S)
    g = np.minimum(sp // GRP, NGRP - 1)
    srel = sp - g * GRP

    key = ((core * NB + b) * NGRP + g) * TPC + t
    order = np.argsort(key, kind="stable")
    cbg = (core * NB + b) * NGRP + g
    cnt_cbg = np.bincount(cbg, minlength=CORES * NB * NGRP).reshape(CORES, NB, NGRP)
    CH = -(-cnt_cbg.max(0) // 128)  # [NB, NGRP] chunks per span
    assert CH.max() * 128 <= 8192
    cnt_cbgt = np.bincount(key, minlength=CORES * NB * NGRP * TPC).reshape(
        CORES, NB, NGRP, TPC
    )

    # span chunk base, block column base (span layout: per block, groups 0..3)
    # gathers split at SPLIT chunks so SWDGE desc-gen never stalls on ring
    # space (8192-idx instructions leave ~12us gaps; 2-4k-idx ones do not)
    span_base = np.zeros((NB, NGRP), np.int64)
    nchunks = 0
    nsplits = 0
    blocks = []
    split_span = []  # split -> (b, g, chunk offset within span, nch)
    for bb in range(NB):
        tiles = list(range(bb * BLK, min((bb + 1) * BLK, TPC)))
        col0 = nchunks
        gathers = []
        for gg in range(NGRP):
            span_base[bb, gg] = nchunks
            ch = int(CH[bb, gg])
            at = 0
            while at < ch:
                nch = min(SPLIT, ch - at)
                gathers.append((gg, nchunks - col0 + at, nch, nsplits))
                split_span.append((bb, gg, at, nch))
                nsplits += 1
                at += nch
            nchunks += ch
        blocks.append(dict(tiles=tiles, C=nchunks - col0, gathers=gathers))
    NC = nchunks
    NSLOT = NC * 128

    # static per-tile chunk lists from min/max tile offsets within spans
    tile_chunks = {tt: [] for tt in range(TPC)}  # (col_in_block, g, chunk_in_span)
    for bb in range(NB):
        tiles = blocks[bb]["tiles"]
        col0 = span_base[bb, 0] if NGRP > 0 else 0
        for gg in range(NGRP):
            c = cnt_cbgt[:, bb, gg, tiles]  # [CORES, ntiles]
            off = np.concatenate(
                [np.zeros((CORES, 1), np.int64), np.cumsum(c, axis=1)], axis=1
            )
            for j, tt in enumerate(tiles):
                lo = int(off[:, j].min()) >> 7
                hi = -(-int(off[:, j + 1].max()) // 128)
                hi = min(hi, int(CH[bb, gg]))
                for k in range(lo, hi):
                    tile_chunks[tt].append(
                        (int(span_base[bb, gg] - col0) + k, gg, k)
                    )

    # per-core slot arrays
    skey = key[order]
    runs = np.flatnonzero(np.diff(skey)) + 1
    starts = np.r_[0, runs]
    lens = np.diff(np.r_[starts, len(skey)])
    # position within the (c,b,g) span: position within run + offset of run's
    # tile within the span
    off_in_span = np.zeros(len(skey), np.int64)
    co_sorted = core[order]
    b_sorted = b[order]
    g_sorted = g[order]
    t_sorted = t[order]
    run_first = starts
    run_tile_off = np.zeros(len(starts), np.int64)
    for i, st in enumerate(starts):
        c_, b_, g_, t_ = (
            co_sorted[st],
            b_sorted[st],
            g_sorted[st],
            t_sorted[st],
        )
        tiles = blocks[b_]["tiles"]
        j = t_ - tiles[0]
        run_tile_off[i] = cnt_cbgt[c_, b_, g_, tiles[0] : tiles[0] + j].sum()
    pos_in_run = np.arange(len(skey)) - np.repeat(starts, lens)
    pos_in_span = pos_in_run + np.repeat(run_tile_off, lens)
    slot = span_base[b_sorted, g_sorted] * 128 + pos_in_span

    pad_val = -1 if PAD_NEG else 0
    idx_slot = np.full((CORES, NSLOT), pad_val, np.int16)
    dr_slot = np.full((CORES, NSLOT), -1, np.int32)
    t_slot = np.full((CORES, NSLOT), -1, np.int32)
    idx_slot[co_sorted, slot] = srel[order].astype(np.int16)
    dr_slot[co_sorted, slot] = r[order]
    t_slot[co_sorted, slot] = t_sorted

    # per-(core, split) valid counts; pads are trailing per span, so a
    # split's valid count is a clamp of (span count - split base)
    cnt_span = cnt_cbg.reshape(CORES, NB * NGRP)
    if PAD_NEG:
        # first two blocks of the program gather their pads (idx 0) so the
        # m-tile double buffers never expose uninitialized SBUF to the PE
        for bb in range(min(2, NB)):
            for gg in range(NGRP):
                s0 = int(span_base[bb, gg]) * 128
                s1 = s0 + int(CH[bb, gg]) * 128
                m = np.zeros(NSLOT, bool)
                m[s0:s1] = True
                idx_slot[m[None, :] & (idx_slot < 0)] = 0
                cnt_span[:, bb * NGRP + gg] = int(CH[bb, gg]) * 128
    nvalid = np.zeros((CORES, len(split_span)), np.int32)
    for si, (bb, gg, at, nch) in enumerate(split_span):
        base = at * 128
        v = np.clip(cnt_span[:, bb * NGRP + gg] - base, 0, nch * 128)
        if PAD_NEG:
            # a core with an all-pad split still needs >=1 valid idx: make
            # the split's first slot a gathered idx-0 pad (P row is zero)
            z = v == 0
            if z.any():
                s0 = (int(span_base[bb, gg]) + at) * 128
                idx_slot[z, s0] = 0
                v = np.maximum(v, 1)
        nvalid[:, si] = v
    return idx_slot, dr_slot, t_slot, nvalid, blocks, tile_chunks, CH, span_base, NC, NSLOT


def _build_bass(blocks, tile_chunks, CH, NC, NSLOT, NPCH):
    import concourse.bacc as bacc
    import concourse.tile as tile
    import concourse.mybir as mybir

    dt = mybir.dt
    nc = bacc.Bacc(
        "TRN2",
        target_bir_lowering=False,
        debug=False,
        num_devices=CORES,
        num_swdge_queues=4,
    )

    xs_in = nc.dram_tensor("xs", [NP, D], dt.float16, kind="ExternalInput")
    xsh_in = nc.dram_tensor("xsh", [SP, D], dt.float16, kind="ExternalInput")
    eye_in = nc.dram_tensor("eye", [128, 128], dt.float16, kind="ExternalInput")
    w1t_in = nc.dram_tensor("w1t", [D, D], dt.float16, kind="ExternalInput")
    w2t_in = nc.dram_tensor("w2t", [D, D], dt.float16, kind="ExternalInput")
    idx_in = nc.dram_tensor("idx", [128, NSLOT // 16], dt.int16, kind="ExternalInput")
    pm_in = nc.dram_tensor("pm", [128, NPCH * 128], dt.float16, kind="ExternalInput")
    nv_in = nc.dram_tensor("nv", [1, NB * NGRP], dt.int32, kind="ExternalInput")
    d1_in = nc.dram_tensor("d1col", [128, TPC], dt.float32, kind="ExternalInput")
    d2_in = nc.dram_tensor("d2col", [128, TPC], dt.float32, kind="ExternalInput")
    c1d_in = nc.dram_tensor("c1d", [SP, D], dt.float16, kind="ExternalInput")
    c2_in = nc.dram_tensor("c2", [SP, D], dt.float32, kind="ExternalInput")
    out_ext = nc.dram_tensor("out", [SP, D], dt.float32, kind="ExternalOutput")

    GBASE = [i * GRP for i in range(NGRP)]
    GLEN = [min(GRP, NP - i * GRP) for i in range(NGRP)]

    # per-tile P offset (tile-major contiguous pidx)
    pofs = {}
    acc = 0
    for tt in range(TPC):
        pofs[tt] = acc
        acc += len(tile_chunks[tt])
    assert acc == NPCH

    with tile.TileContext(nc) as tc:
        with (
            tc.tile_pool(name="const", bufs=1) as cpool,
            tc.tile_pool(name="mblk", bufs=3) as mpool,
            tc.tile_pool(name="pblk", bufs=3) as ppool,
            tc.tile_pool(name="gs", bufs=4) as gspool,
            tc.tile_pool(name="ytmp", bufs=4) as ytpool,
            tc.tile_pool(name="cload", bufs=4) as clpool,
            tc.tile_pool(name="psumG", bufs=4, space="PSUM") as pgpool,
            tc.tile_pool(name="psumH", bufs=4, space="PSUM") as phpool,
            tc.tile_pool(name="dram", bufs=1, space="DRAM") as dram_pool,
        ):
            w1t_t = cpool.tile([D, D], dt.float16)
            nc.sync.dma_start(out=w1t_t[:], in_=w1t_in[:, :])
            w2t_t = cpool.tile([D, D], dt.float16)
            nc.sync.dma_start(out=w2t_t[:], in_=w2t_in[:, :])
            idx_t = cpool.tile([128, NSLOT // 16], dt.int16)
            nc.sync.dma_start(out=idx_t[:], in_=idx_in[:, :])
            nv_t = cpool.tile([1, NB * NGRP], dt.int32)
            nc.sync.dma_start(out=nv_t[:], in_=nv_in[:, :])
            d1_t = cpool.tile([128, TPC], dt.float32)
            nc.sync.dma_start(out=d1_t[:], in_=d1_in[:, :])
            d2_t = cpool.tile([128, TPC], dt.float32)
            nc.sync.dma_start(out=d2_t[:], in_=d2_in[:, :])
            eye_t = cpool.tile([128, 128], dt.float16)
            nc.sync.dma_start(out=eye_t[:], in_=eye_in[:, :])

            # even split measured best: a later/bigger early half does not
            # finish inside its overlap window and regresses
            HALF = (TPC // 2 + 1) * 128  # 6400 rows = tiles 0-49
            y1s0 = dram_pool.tile([HALF, D], dt.float16)
            y1s1 = dram_pool.tile([SP - HALF, D], dt.float16)
            y1h0 = dram_pool.tile([CORES * HALF, D], dt.float16)
            y1h1 = dram_pool.tile([CORES * (SP - HALF), D], dt.float16)
            y1_full = dram_pool.tile([NP, D], dt.float16)

            def layer(src_dram, selfsh, wt_t, last, post_block=None):
                for bb, blk in enumerate(blocks):
                    C = blk["C"]
                    m_t = mpool.tile([128, C, D], dt.float16, tag="m")
                    for gg, cofs, nch, _spl in blk["gathers"]:
                        if nch == 0:
                            continue
                        # m_t columns [cofs, cofs+nch); pads are idx 0 so
                        # every slot is gathered (num_idxs_reg == num)
                        num = nch * 128
                        s0 = (sum(b2["C"] for b2 in blocks[:bb]) + cofs) * 128
                        nc.gpsimd.dma_gather(
                            m_t[:, cofs : cofs + nch, :],
                            src_dram[GBASE[gg] : GBASE[gg] + GLEN[gg], :],
                            idx_t[:, s0 // 16 : (s0 + num) // 16],
                            num,
                            num,
                            D,
                            single_packet=False,
                            # alternate SWDGE queues: 4 desc-gen contexts
                            # exist in ucode; two queues may overlap gen
                            queue_num=_spl % 4,
                        )
                    for tt in blk["tiles"]:
                        chunks = tile_chunks[tt]
                        nchk = len(chunks)
                        p_t = ppool.tile([128, nchk, 128], dt.float16, tag="p")
                        o0 = pofs[tt] * 128
                        nc.sync.dma_start(
                            out=p_t[:], in_=pm_in[:, o0 : o0 + nchk * 128]
                        )
                        rows = slice(tt * 128, (tt + 1) * 128)
                        psum_g = pgpool.tile([128, 128], dt.float32, space="PSUM")
                        for i, (col, _gg, _k) in enumerate(chunks):
                            nc.tensor.matmul(
                                psum_g[:],
                                lhsT=m_t[:, col, :],
                                rhs=p_t[:, i, :],
                                start=(i == 0),
                                stop=False,
                            )
                        # self-loop term: agg^T += x_tile^T via one identity
                        # matmul on a plain bulk load of the own shard tile
                        # (self-loops never occupy gather slots)
                        xt_t = clpool.tile([128, 128], dt.float16, tag="xt")
                        sten, soff = selfsh(tt)
                        nc.sync.dma_start(
                            out=xt_t[:],
                            in_=sten[tt * 128 - soff : (tt + 1) * 128 - soff, :],
                        )
                        nc.tensor.matmul(
                            psum_g[:],
                            lhsT=xt_t[:],
                            rhs=eye_t[:],
                            start=False,
                            stop=True,
                        )
                        gs_t = gspool.tile([128, 128], dt.float16, tag="gs")
                        nc.scalar.copy(out=gs_t[:], in_=psum_g[:])
                        psum_h = phpool.tile([128, 128], dt.float32, space="PSUM")
                        nc.tensor.matmul(
                            psum_h[:], lhsT=gs_t[:], rhs=wt_t[:], start=True, stop=True
                        )
                        if not last:
                            tmp_t = ytpool.tile([128, 128], dt.float16, tag="yt")
                            nc.scalar.mul(tmp_t[:], psum_h[:], d1_t[:, tt : tt + 1])
                            c1_t = clpool.tile([128, 128], dt.float16, tag="c1")
                            nc.sync.dma_start(out=c1_t[:], in_=c1d_in[rows, :])
                            y1_t = ytpool.tile([128, 128], dt.float16, tag="y1")
                            nc.vector.tensor_tensor(
                                out=y1_t[:],
                                in0=tmp_t[:],
                                in1=c1_t[:],
                                op=mybir.AluOpType.add,
                            )
                            if tt * 128 < HALF:
                                nc.sync.dma_start(
                                    out=y1s0[rows, :], in_=y1_t[:]
                                )
                            else:
                                nc.sync.dma_start(
                                    out=y1s1[
                                        tt * 128 - HALF : (tt + 1) * 128 - HALF, :
                                    ],
                                    in_=y1_t[:],
                                )
                        else:
                            tmp_t = ytpool.tile([128, 128], dt.float32, tag="yt2")
                            nc.scalar.mul(tmp_t[:], psum_h[:], d2_t[:, tt : tt + 1])
                            c2_t = clpool.tile([128, 128], dt.float32, tag="c2")
                            nc.sync.dma_start(out=c2_t[:], in_=c2_in[rows, :])
                            o_t = ytpool.tile([128, 128], dt.float32, tag="o")
                            nc.vector.tensor_tensor(
                                out=o_t[:],
                                in0=tmp_t[:],
                                in1=c2_t[:],
                                op=mybir.AluOpType.add,
                            )
                            nc.sync.dma_start(out=out_ext[rows, :], in_=o_t[:])
                    if post_block is not None:
                        post_block(bb)

            # AllGather in two halves via whole-tile collectives (sliced or
            # strided collective APs fail to compile): half 0 (tiles 0-49)
            # is issued as soon as block 6 completes and its gather result
            # is re-assembled into y1_full layout by 8 contiguous DRAM
            # copies, all overlapping the rest of layer 1. Only half 1
            # remains on the layer boundary.
            def ag_half(shard, gathered, lo, hi):
                nc.gpsimd.collective_compute(
                    "AllGather",
                    mybir.AluOpType.bypass,
                    replica_groups=[list(range(CORES))],
                    ins=[shard.opt()],
                    outs=[gathered.opt()],
                )
                ln = hi - lo
                for c in range(CORES):
                    nc.sync.dma_start(
                        out=y1_full[c * SP + lo : c * SP + hi, :],
                        in_=gathered[c * ln : (c + 1) * ln, :],
                    )

            def maybe_ag(bb):
                if bb == 6:
                    ag_half(y1s0, y1h0, 0, HALF)

            def self1(tt):
                return xsh_in, 0

            def self2(tt):
                return (y1s0, 0) if tt * 128 < HALF else (y1s1, HALF)

            layer(xs_in, self1, w1t_t, last=False, post_block=maybe_ag)
            ag_half(y1s1, y1h1, HALF, SP)
            layer(y1_full, self2, w2t_t, last=True)

    nc.compile()
    return nc


def _prepare(x, edge_index, perturb_first, perturb_last, W1, b1, W2, b2):
    x = np.asarray(x, np.float32)
    edge_index = np.asarray(edge_index)
    # deg includes the self-loop (+1); the schedule excludes self-loops (they
    # are applied via a per-tile transposed bulk load, not gather slots)
    src = edge_index[0].astype(np.int64)
    dst = edge_index[1].astype(np.int64)
    deg = np.bincount(dst, minlength=N).astype(np.float32) + 1.0
    dinv = 1.0 / np.sqrt(deg)

    (
        idx_slot,
        dr_slot,
        t_slot,
        nvalid,
        blocks,
        tile_chunks,
        CH,
        span_base,
        NC,
        NSLOT,
    ) = _build_schedule(src, dst)

    NPCH = sum(len(tile_chunks[tt]) for tt in range(TPC))

    # (tile, g, chunk_in_span) -> pidx
    M = np.full((TPC, NGRP, int(CH.max())), -1, np.int64)
    acc = 0
    for tt in range(TPC):
        for (col, gg, k) in tile_chunks[tt]:
            M[tt, gg, k] = acc
            acc += 1

    xs = np.zeros((NP, D), np.float16)
    dinv_x = (dinv[:, None] * x).astype(np.float16)
    for c in range(CORES):
        xs[c * SP : c * SP + S] = dinv_x[c * S : (c + 1) * S]

    w1t = np.asarray(W1, np.float32).T.astype(np.float16).copy()
    w2t = np.asarray(W2, np.float32).T.astype(np.float16).copy()

    c1 = np.asarray(perturb_first, np.float32) + np.asarray(b1, np.float32)[None, :]
    c1d = dinv[:, None] * c1
    c2 = np.asarray(perturb_last, np.float32) + np.asarray(b2, np.float32)[None, :]

    sl = np.arange(NSLOT)
    chunkid = sl >> 7
    slotin = sl & 127
    # chunk -> (g, chunk_in_span) lookup
    ch_g = np.zeros(NC, np.int64)
    ch_k = np.zeros(NC, np.int64)
    for bb in range(NB):
        for gg in range(NGRP):
            b0 = int(span_base[bb, gg])
            n = int(CH[bb, gg])
            ch_g[b0 : b0 + n] = gg
            ch_k[b0 : b0 + n] = np.arange(n)

    in_maps = []
    for c in range(CORES):
        rows = slice(c * S, (c + 1) * S)
        c1d_p = np.zeros((SP, D), np.float16)
        c1d_p[:S] = c1d[rows].astype(np.float16)
        c2_p = np.zeros((SP, D), np.float32)
        c2_p[:S] = c2[rows]
        dcol = np.zeros((TPC * 128,), np.float32)
        dcol[:S] = dinv[rows]
        idx_l = np.tile(idx_slot[c].reshape(-1, 16).T, (8, 1)).copy()

        P = np.zeros((NPCH, 128, 128), np.float16)
        dr = dr_slot[c]
        ts = t_slot[c]
        v = dr >= 0
        pidx = M[ts[v], ch_g[chunkid[v]], ch_k[chunkid[v]]]
        assert (pidx >= 0).all()
        P[pidx, slotin[v], dr[v]] = 1.0
        pm = np.ascontiguousarray(P.transpose(1, 0, 2).reshape(128, NPCH * 128))
        in_maps.append(
            {
                "xs": xs,
                "xsh": xs[c * SP : (c + 1) * SP],
                "eye": np.eye(128, dtype=np.float16),
                "w1t": w1t,
                "w2t": w2t,
                "idx": idx_l,
                "pm": pm,
                "nv": nvalid[c : c + 1],
                "d1col": np.ascontiguousarray((dcol ** 2).reshape(TPC, 128).T),
                "d2col": np.ascontiguousarray(dcol.reshape(TPC, 128).T),
                "c1d": c1d_p,
                "c2": c2_p,
            }
        )
    return in_maps, blocks, tile_chunks, CH, NC, NSLOT, NPCH


def kernel(
    x,
    edge_index,
    perturb_first,
    perturb_last,
    W1,
    b1,
    W2,
    b2,
    _results=[],
    _trace=False,
    _tmpdir=None,
):
    from concourse.bass_utils import run_bass_kernel_spmd

    in_maps, blocks, tile_chunks, CH, NC, NSLOT, NPCH = _prepare(
        x, edge_index, perturb_first, perturb_last, W1, b1, W2, b2
    )
    nc = _build_bass(blocks, tile_chunks, CH, NC, NSLOT, NPCH)
    res = run_bass_kernel_spmd(
        nc, in_maps, core_ids=list(range(CORES)), trace=_trace, tmpdir=_tmpdir
    )
    _results.append(res)
    out = np.concatenate([res.results[c]["out"][:S] for c in range(CORES)], axis=0)
    return out.astype(np.float32)


# revision 44
# speedup vs baseline: 1.0317x; 1.0317x over previous
"""Trainium2 Bass kernel for a 2-layer GCN (PyG GCNConv semantics) on 8 NeuronCores.

Design. The dominant cost is the per-edge dma_gather: Q7 SWDGE descriptor
generation runs at ~7.9ns/index per queue, but 4 SWDGE queues (queue_num
round-robin + num_swdge_queues=4) overlap generation to ~2.5ns/index,
leaving the 16 SDMA engines' 256B-descriptor drain as the floor:

  - nodes sharded 12500/core (12544 padded = 98 tiles of 128 rows); edges
    partitioned by dst core, grouped by (dst-block of 8 tiles, src-group),
    sorted by dst tile within each (block, group) span
  - spans are chunked into 128-slot chunks with NO per-tile alignment: a
    chunk at a tile boundary is consumed by both tiles (each with its own
    host-precomputed one-hot P slice; other tiles' slots are zero rows).
    Pads are gathered idx-0 rows with zero P rows (HW-measured: trailing -1
    pads desc-gen SLOWER, and runtime num_idxs_reg wedges the device)
  - gathers split at 4096 idx: larger instructions stall the SWDGE ring
  - self-loops never occupy gather slots (their src group depends on the
    core, which inflates the max-over-cores span pad AND they need no
    gather): applied per tile as one PE matmul of the bulk-loaded own-shard
    tile against an identity rhs, accumulated into the same PSUM chain
  - per chunk: PE matmul accumulates agg^T[feat, dstrow] into PSUM
  - per dst-tile: agg^T @ W^T; dinv[dst] applied post-matmul on ACT
  - norm separability: dinv[src] pre-scaled into gather rows, dinv[dst]
    post-applied (layer 1 uses dinv^2: the extra dinv is layer 2's source
    factor)
  - AllGather of y1 split in two halves: the first is issued mid-layer-1
    and re-assembled into y1_full by contiguous DRAM copies, so only half
    the collective sits on the layer boundary

Engine discipline: no DVE tensor_scalar/copy (2-port perf mode blocks the
SWDGE descriptor generation that dominates runtime). PSUM evacuation +
scaling on ACT; adds on DVE tensor_tensor (single-port)."""
import sys

sys.path.insert(0, "/opt/trn_rl_repo")

import numpy as np

N = 100000
E = 1600000
D = 128
CORES = 8
S = 12500          # real nodes per core
TPC = 98           # dst tiles per core
SP = TPC * 128     # padded nodes per core (12544)
NP = CORES * SP    # padded global rows (100352)
GRP = 32768        # src-group width (int16 gather-index limit)
NGRP = 4
BLK = 8            # dst tiles per block
NB = (TPC + BLK - 1) // BLK
PAD_NEG = False    # HW-measured: trailing -1 pads cost MORE gen time than
                   # gathered idx-0 pads, and value_load num_idxs_reg wedges
                   # the device -- so pads are idx 0, gathered, zero P rows
SPLIT = 8          # chunks per gather instruction (1024 idx): finer
                   # round-robin over the 4 SWDGE queues


def _build_schedule(src, dst):
    core = dst // S
    dl = dst % S
    t = dl >> 7
    r = dl & 127
    b = t // BLK
    sp = (src // S) * SP + (src % S)
    g = np.minimum(sp // GRP, NGRP - 1)
    srel = sp - g * GRP

    key = ((core * NB + b) * NGRP + g) * TPC + t
    order = np.argsort(key, kind="stable")
    cbg = (core * NB + b) * NGRP + g
    cnt_cbg = np.bincount(cbg, minlength=CORES * NB * NGRP).reshape(CORES, NB, NGRP)
    CH = -(-cnt_cbg.max(0) // 128)  # [NB, NGRP] chunks per span
    assert CH.max() * 128 <= 8192
    cnt_cbgt = np.bincount(key, minlength=CORES * NB * NGRP * TPC).reshape(
        CORES, NB, NGRP, TPC
    )

    # span chunk base, block column base (span layout: per block, groups 0..3)
    # gathers split at SPLIT chunks so SWDGE desc-gen never stalls on ring
    # space (8192-idx instructions leave ~12us gaps; 2-4k-idx ones do not)
    span_base = np.zeros((NB, NGRP), np.int64)
    nchunks = 0
    nsplits = 0
    blocks = []
    split_span = []  # split -> (b, g, chunk offset within span, nch)
    for bb in range(NB):
        tiles = list(range(bb * BLK, min((bb + 1) * BLK, TPC)))
        col0 = nchunks
        gathers = []
        for gg in range(NGRP):
            span_base[bb, gg] = nchunks
            ch = int(CH[bb, gg])
            at = 0
            while at < ch:
                nch = min(SPLIT, ch - at)
                gathers.append((gg, nchunks - col0 + at, nch, nsplits))
                split_span.append((bb, gg, at, nch))
                nsplits += 1
                at += nch
            nchunks += ch
        blocks.append(dict(tiles=tiles, C=nchunks - col0, gathers=gathers))
    NC = nchunks
    NSLOT = NC * 128

    # static per-tile chunk lists from min/max tile offsets within spans
    tile_chunks = {tt: [] for tt in range(TPC)}  # (col_in_block, g, chunk_in_span)
    for bb in range(NB):
        tiles = blocks[bb]["tiles"]
        col0 = span_base[bb, 0] if NGRP > 0 else 0
        for gg in range(NGRP):
            c = cnt_cbgt[:, bb, gg, tiles]  # [CORES, ntiles]
            off = np.concatenate(
                [np.zeros((CORES, 1), np.int64), np.cumsum(c, axis=1)], axis=1
            )
            for j, tt in enumerate(tiles):
                lo = int(off[:, j].min()) >> 7
                hi = -(-int(off[:, j + 1].max()) // 128)
                hi = min(hi, int(CH[bb, gg]))
                for k in range(lo, hi):
                    tile_chunks[tt].append(
                        (int(span_base[bb, gg] - col0) + k, gg, k)
                    )

    # per-core slot arrays
    skey = key[order]
    runs = np.flatnonzero(np.diff(skey)) + 1
    starts = np.r_[0, runs]
    lens = np.diff(np.r_[starts, len(skey)])
    # position within the (c,b,g) span: position within run + offset of run's
    # tile within the span
    off_in_span = np.zeros(len(skey), np.int64)
    co_sorted = core[order]
    b_sorted = b[order]
    g_sorted = g[order]
    t_sorted = t[order]
    run_first = starts
    run_tile_off = np.zeros(len(starts), np.int64)
    for i, st in enumerate(starts):
        c_, b_, g_, t_ = (
            co_sorted[st],
            b_sorted[st],
            g_sorted[st],
            t_sorted[st],
        )
        tiles = blocks[b_]["tiles"]
        j = t_ - tiles[0]
        run_tile_off[i] = cnt_cbgt[c_, b_, g_, tiles[0] : tiles[0] + j].sum()
    pos_in_run = np.arange(len(skey)) - np.repeat(starts, lens)
    pos_in_span = pos_in_run + np.repeat(run_tile_off, lens)
    slot = span_base[b_sorted, g_sorted] * 128 + pos_in_span

    pad_val = -1 if PAD_NEG else 0
    idx_slot = np.full((CORES, NSLOT), pad_val, np.int16)
    dr_slot = np.full((CORES, NSLOT), -1, np.int32)
    t_slot = np.full((CORES, NSLOT), -1, np.int32)
    idx_slot[co_sorted, slot] = srel[order].astype(np.int16)
    dr_slot[co_sorted, slot] = r[order]
    t_slot[co_sorted, slot] = t_sorted

    # per-(core, split) valid counts; pads are trailing per span, so a
    # split's valid count is a clamp of (span count - split base)
    cnt_span = cnt_cbg.reshape(CORES, NB * NGRP)
    if PAD_NEG:
        # first two blocks of the program gather their pads (idx 0) so the
        # m-tile double buffers never expose uninitialized SBUF to the PE
        for bb in range(min(2, NB)):
            for gg in range(NGRP):
                s0 = int(span_base[bb, gg]) * 128
                s1 = s0 + int(CH[bb, gg]) * 128
                m = np.zeros(NSLOT, bool)
                m[s0:s1] = True
                idx_slot[m[None, :] & (idx_slot < 0)] = 0
                cnt_span[:, bb * NGRP + gg] = int(CH[bb, gg]) * 128
    nvalid = np.zeros((CORES, len(split_span)), np.int32)
    for si, (bb, gg, at, nch) in enumerate(split_span):
        base = at * 128
        v = np.clip(cnt_span[:, bb * NGRP + gg] - base, 0, nch * 128)
        if PAD_NEG:
            # a core with an all-pad split still needs >=1 valid idx: make
            # the split's first slot a gathered idx-0 pad (P row is zero)
            z = v == 0
            if z.any():
                s0 = (int(span_base[bb, gg]) + at) * 128
                idx_slot[z, s0] = 0
                v = np.maximum(v, 1)
        nvalid[:, si] = v
    return idx_slot, dr_slot, t_slot, nvalid, blocks, tile_chunks, CH, span_base, NC, NSLOT


def _build_bass(blocks, tile_chunks, CH, NC, NSLOT, NPCH):
    import concourse.bacc as bacc
    import concourse.tile as tile
    import concourse.mybir as mybir

    dt = mybir.dt
    nc = bacc.Bacc(
        "TRN2",
        target_bir_lowering=False,
        debug=False,
        num_devices=CORES,
        num_swdge_queues=4,
    )

    xs_in = nc.dram_tensor("xs", [NP, D], dt.float16, kind="ExternalInput")
    xsh_in = nc.dram_tensor("xsh", [SP, D], dt.float16, kind="ExternalInput")
    eye_in = nc.dram_tensor("eye", [128, 128], dt.float16, kind="ExternalInput")
    w1t_in = nc.dram_tensor("w1t", [D, D], dt.float16, kind="ExternalInput")
    w2t_in = nc.dram_tensor("w2t", [D, D], dt.float16, kind="ExternalInput")
    idx_in = nc.dram_tensor("idx", [128, NSLOT // 16], dt.int16, kind="ExternalInput")
    pm_in = nc.dram_tensor("pm", [128, NPCH * 128], dt.float16, kind="ExternalInput")
    nv_in = nc.dram_tensor("nv", [1, NB * NGRP], dt.int32, kind="ExternalInput")
    d1_in = nc.dram_tensor("d1col", [128, TPC], dt.float32, kind="ExternalInput")
    d2_in = nc.dram_tensor("d2col", [128, TPC], dt.float32, kind="ExternalInput")
    c1d_in = nc.dram_tensor("c1d", [SP, D], dt.float16, kind="ExternalInput")
    c2_in = nc.dram_tensor("c2", [SP, D], dt.float32, kind="ExternalInput")
    out_ext = nc.dram_tensor("out", [SP, D], dt.float32, kind="ExternalOutput")

    GBASE = [i * GRP for i in range(NGRP)]
    GLEN = [min(GRP, NP - i * GRP) for i in range(NGRP)]

    # per-tile P offset (tile-major contiguous pidx)
    pofs = {}
    acc = 0
    for tt in range(TPC):
        pofs[tt] = acc
        acc += len(tile_chunks[tt])
    assert acc == NPCH

    with tile.TileContext(nc) as tc:
        with (
            tc.tile_pool(name="const", bufs=1) as cpool,
            tc.tile_pool(name="mblk", bufs=2) as mpool,
            tc.tile_pool(name="pblk", bufs=3) as ppool,
            tc.tile_pool(name="gs", bufs=4) as gspool,
            tc.tile_pool(name="ytmp", bufs=4) as ytpool,
            tc.tile_pool(name="cload", bufs=4) as clpool,
            tc.tile_pool(name="psumG", bufs=4, space="PSUM") as pgpool,
            tc.tile_pool(name="psumH", bufs=4, space="PSUM") as phpool,
            tc.tile_pool(name="dram", bufs=1, space="DRAM") as dram_pool,
        ):
            w1t_t = cpool.tile([D, D], dt.float16)
            nc.sync.dma_start(out=w1t_t[:], in_=w1t_in[:, :])
            w2t_t = cpool.tile([D, D], dt.float16)
            nc.sync.dma_start(out=w2t_t[:], in_=w2t_in[:, :])
            idx_t = cpool.tile([128, NSLOT // 16], dt.int16)
            nc.sync.dma_start(out=idx_t[:], in_=idx_in[:, :])
            nv_t = cpool.tile([1, NB * NGRP], dt.int32)
            nc.sync.dma_start(out=nv_t[:], in_=nv_in[:, :])
            d1_t = cpool.tile([128, TPC], dt.float32)
            nc.sync.dma_start(out=d1_t[:], in_=d1_in[:, :])
            d2_t = cpool.tile([128, TPC], dt.float32)
            nc.sync.dma_start(out=d2_t[:], in_=d2_in[:, :])
            eye_t = cpool.tile([128, 128], dt.float16)
            nc.sync.dma_start(out=eye_t[:], in_=eye_in[:, :])

            # even split measured best: a later/bigger early half does not
            # finish inside its overlap window and regresses
            HALF = (TPC // 2 + 1) * 128  # 6400 rows = tiles 0-49
            y1s0 = dram_pool.tile([HALF, D], dt.float16)
            y1s1 = dram_pool.tile([SP - HALF, D], dt.float16)
            y1h0 = dram_pool.tile([CORES * HALF, D], dt.float16)
            y1h1 = dram_pool.tile([CORES * (SP - HALF), D], dt.float16)
            y1_full = dram_pool.tile([NP, D], dt.float16)

            def layer(src_dram, selfsh, wt_t, last, post_block=None):
                for bb, blk in enumerate(blocks):
                    C = blk["C"]
                    m_t = mpool.tile([128, C, D], dt.float16, tag="m")
                    for gg, cofs, nch, _spl in blk["gathers"]:
                        if nch == 0:
                            continue
                        # m_t columns [cofs, cofs+nch); pads are idx 0 so
                        # every slot is gathered (num_idxs_reg == num)
                        num = nch * 128
                        s0 = (sum(b2["C"] for b2 in blocks[:bb]) + cofs) * 128
                        nc.gpsimd.dma_gather(
                            m_t[:, cofs : cofs + nch, :],
                            src_dram[GBASE[gg] : GBASE[gg] + GLEN[gg], :],
                            idx_t[:, s0 // 16 : (s0 + num) // 16],
                            num,
                            num,
                            D,
                            single_packet=False,
                            # alternate SWDGE queues: 4 desc-gen contexts
                            # exist in ucode; two queues may overlap gen
                            queue_num=_spl % 4,
                        )
                    for tt in blk["tiles"]:
                        chunks = tile_chunks[tt]
                        nchk = len(chunks)
                        p_t = ppool.tile([128, nchk, 128], dt.float16, tag="p")
                        o0 = pofs[tt] * 128
                        nc.sync.dma_start(
                            out=p_t[:], in_=pm_in[:, o0 : o0 + nchk * 128]
                        )
                        rows = slice(tt * 128, (tt + 1) * 128)
                        psum_g = pgpool.tile([128, 128], dt.float32, space="PSUM")
                        for i, (col, _gg, _k) in enumerate(chunks):
                            nc.tensor.matmul(
                                psum_g[:],
                                lhsT=m_t[:, col, :],
                                rhs=p_t[:, i, :],
                                start=(i == 0),
                                stop=False,
                            )
                        # self-loop term: agg^T += x_tile^T via one identity
                        # matmul on a plain bulk load of the own shard tile
                        # (self-loops never occupy gather slots)
                        xt_t = clpool.tile([128, 128], dt.float16, tag="xt")
                        sten, soff = selfsh(tt)
                        nc.sync.dma_start(
                            out=xt_t[:],
                            in_=sten[tt * 128 - soff : (tt + 1) * 128 - soff, :],
                        )
                        nc.tensor.matmul(
                            psum_g[:],
                            lhsT=xt_t[:],
                            rhs=eye_t[:],
                            start=False,
                            stop=True,
                        )
                        gs_t = gspool.tile([128, 128], dt.float16, tag="gs")
                        nc.scalar.copy(out=gs_t[:], in_=psum_g[:])
                        psum_h = phpool.tile([128, 128], dt.float32, space="PSUM")
                        nc.tensor.matmul(
                            psum_h[:], lhsT=gs_t[:], rhs=wt_t[:], start=True, stop=True
                        )
                        if not last:
                            tmp_t = ytpool.tile([128, 128], dt.float16, tag="yt")
                            nc.scalar.mul(tmp_t[:], psum_h[:], d1_t[:, tt : tt + 1])
                            c1_t = clpool.tile([128, 128], dt.float16, tag="c1")
                            nc.sync.dma_start(out=c1_t[:], in_=c1d_in[rows, :])
                            y1_t = ytpool.tile([128, 128], dt.float16, tag="y1")
                            nc.vector.tensor_tensor(
                                out=y1_t[:],
                                in0=tmp_t[:],
                                in1=c1_t[:],
                                op=mybir.AluOpType.add,
                            )
                            if tt * 128 < HALF:
                                nc.sync.dma_start(
                                    out=y1s0[rows, :], in_=y1_t[:]
                                )
                            else:
                                nc.sync.dma_start(
                                    out=y1s1[
                                        tt * 128 - HALF : (tt + 1) * 128 - HALF, :
                                    ],
                                    in_=y1_t[:],
                                )
                        else:
                            tmp_t = ytpool.tile([128, 128], dt.float32, tag="yt2")
                            nc.scalar.mul(tmp_t[:], psum_h[:], d2_t[:, tt : tt + 1])
                            c2_t = clpool.tile([128, 128], dt.float32, tag="c2")
                            nc.sync.dma_start(out=c2_t[:], in_=c2_in[rows, :])
                            o_t = ytpool.tile([128, 128], dt.float32, tag="o")
                            nc.vector.tensor_tensor(
                                out=o_t[:],
                                in0=tmp_t[:],
                                in1=c2_t[:],
                                op=mybir.AluOpType.add,
                            )
                            nc.sync.dma_start(out=out_ext[rows, :], in_=o_t[:])
                    if post_block is not None:
                        post_block(bb)

            # AllGather in two halves via whole-tile collectives (sliced or
            # strided collective APs fail to compile): half 0 (tiles 0-49)
            # is issued as soon as block 6 completes and its gather result
            # is re-assembled into y1_full layout by 8 contiguous DRAM
            # copies, all overlapping the rest of layer 1. Only half 1
            # remains on the layer boundary.
            def ag_half(shard, gathered, lo, hi):
                nc.gpsimd.collective_compute(
                    "AllGather",
                    mybir.AluOpType.bypass,
                    replica_groups=[list(range(CORES))],
                    ins=[shard.opt()],
                    outs=[gathered.opt()],
                )
                ln = hi - lo
                for c in range(CORES):
                    nc.sync.dma_start(
                        out=y1_full[c * SP + lo : c * SP + hi, :],
                        in_=gathered[c * ln : (c + 1) * ln, :],
                    )

            def maybe_ag(bb):
                if bb == 6:
                    ag_half(y1s0, y1h0, 0, HALF)

            def self1(tt):
                return xsh_in, 0

            def self2(tt):
                return (y1s0, 0) if tt * 128 < HALF else (y1s1, HALF)

            layer(xs_in, self1, w1t_t, last=False, post_block=maybe_ag)
            ag_half(y1s1, y1h1, HALF, SP)
            layer(y1_full, self2, w2t_t, last=True)

    nc.compile()
    return nc


def _prepare(x, edge_index, perturb_first, perturb_last, W1, b1, W2, b2):
    x = np.asarray(x, np.float32)
    edge_index = np.asarray(edge_index)
    # deg includes the self-loop (+1); the schedule excludes self-loops (they
    # are applied via a per-tile transposed bulk load, not gather slots)
    src = edge_index[0].astype(np.int64)
    dst = edge_index[1].astype(np.int64)
    deg = np.bincount(dst, minlength=N).astype(np.float32) + 1.0
    dinv = 1.0 / np.sqrt(deg)

    (
        idx_slot,
        dr_slot,
        t_slot,
        nvalid,
        blocks,
        tile_chunks,
        CH,
        span_base,
        NC,
        NSLOT,
    ) = _build_schedule(src, dst)

    NPCH = sum(len(tile_chunks[tt]) for tt in range(TPC))

    # (tile, g, chunk_in_span) -> pidx
    M = np.full((TPC, NGRP, int(CH.max())), -1, np.int64)
    acc = 0
    for tt in range(TPC):
        for (col, gg, k) in tile_chunks[tt]:
            M[tt, gg, k] = acc
            acc += 1

    xs = np.zeros((NP, D), np.float16)
    dinv_x = (dinv[:, None] * x).astype(np.float16)
    for c in range(CORES):
        xs[c * SP : c * SP + S] = dinv_x[c * S : (c + 1) * S]

    w1t = np.asarray(W1, np.float32).T.astype(np.float16).copy()
    w2t = np.asarray(W2, np.float32).T.astype(np.float16).copy()

    c1 = np.asarray(perturb_first, np.float32) + np.asarray(b1, np.float32)[None, :]
    c1d = dinv[:, None] * c1
    c2 = np.asarray(perturb_last, np.float32) + np.asarray(b2, np.float32)[None, :]

    sl = np.arange(NSLOT)
    chunkid = sl >> 7
    slotin = sl & 127
    # chunk -> (g, chunk_in_span) lookup
    ch_g = np.zeros(NC, np.int64)
    ch_k = np.zeros(NC, np.int64)
    for bb in range(NB):
        for gg in range(NGRP):
            b0 = int(span_base[bb, gg])
            n = int(CH[bb, gg])
            ch_g[b0 : b0 + n] = gg
            ch_k[b0 : b0 + n] = np.arange(n)

    in_maps = []
    for c in range(CORES):
        rows = slice(c * S, (c + 1) * S)
        c1d_p = np.zeros((SP, D), np.float16)
        c1d_p[:S] = c1d[rows].astype(np.float16)
        c2_p = np.zeros((SP, D), np.float32)
        c2_p[:S] = c2[rows]
        dcol = np.zeros((TPC * 128,), np.float32)
        dcol[:S] = dinv[rows]
        idx_l = np.tile(idx_slot[c].reshape(-1, 16).T, (8, 1)).copy()

        P = np.zeros((NPCH, 128, 128), np.float16)
        dr = dr_slot[c]
        ts = t_slot[c]
        v = dr >= 0
        pidx = M[ts[v], ch_g[chunkid[v]], ch_k[chunkid[v]]]
        assert (pidx >= 0).all()
        P[pidx, slotin[v], dr[v]] = 1.0
        pm = np.ascontiguousarray(P.transpose(1, 0, 2).reshape(128, NPCH * 128))
        in_maps.append(
            {
                "xs": xs,
                "xsh": xs[c * SP : (c + 1) * SP],
                "eye": np.eye(128, dtype=np.float16),
                "w1t": w1t,
                "w2t": w2t,
                "idx": idx_l,
                "pm": pm,
                "nv": nvalid[c : c + 1],
                "d1col": np.ascontiguousarray((dcol ** 2).reshape(TPC, 128).T),
                "d2col": np.ascontiguousarray(dcol.reshape(TPC, 128).T),
                "c1d": c1d_p,
                "c2": c2_p,
            }
        )
    return in_maps, blocks, tile_chunks, CH, NC, NSLOT, NPCH


def kernel(
    x,
    edge_index,
    perturb_first,
    perturb_last,
    W1,
    b1,
    W2,
    b2,
    _results=[],
    _trace=False,
    _tmpdir=None,
):
    from concourse.bass_utils import run_bass_kernel_spmd

    in_maps, blocks, tile_chunks, CH, NC, NSLOT, NPCH = _prepare(
        x, edge_index, perturb_first, perturb_last, W1, b1, W2, b2
    )
    nc = _build_bass(blocks, tile_chunks, CH, NC, NSLOT, NPCH)
    res = run_bass_kernel_spmd(
        nc, in_maps, core_ids=list(range(CORES)), trace=_trace, tmpdir=_tmpdir
    )
    _results.append(res)
    out = np.concatenate([res.results[c]["out"][:S] for c in range(CORES)], axis=0)
    return out.astype(np.float32)
